# revision 14
# baseline (speedup 1.0000x reference)
"""Trainium2 Bass kernel for nn_EssentialMatixModule.

Dual-softmax cross-attention (LoFTR-style) + bilinear feature + projection.
Data-parallel over batch across 8 cores; proj output-sharded with chunked
AllGathers of the (bf16) feature matrix overlapping the attention phase.

v2: software-pipelined attention main loop (1-block skew) so the PE queue
never head-blocks on ACT/DVE results; column sums of E via 1-column
matmuls with E stationary (kills the DVE transpose/extract path); 1/Zr
folded into the E^2 op via scalar_tensor_tensor; x tiles resident in SBUF
(no second load); no DMAs on the scalar queue.
"""

import sys

sys.path.insert(0, "/opt/trn_rl_repo")

from contextlib import ExitStack

import ml_dtypes
import numpy as np

import concourse.bass as bass
import concourse.tile as tile
from concourse import bacc, mybir
from concourse.bass_utils import run_bass_kernel_spmd

B, C, HG, WG = 64, 256, 24, 24
N = HG * WG  # 576
H, HD = 3, 64
F = H * HD  # 192
SCALE = HD**-0.5
EPS = 1e-5
NCORES = 8
BP = B // NCORES  # 8 items per core
NT = [128, 128, 128, 128, 64]  # token tiles (sum=576)
NCH = [(0, 512), (512, 64)]  # free-dim chunks for N=576 psum
DE = 70  # hd + 6 pos dims
PADMH = 4992  # 39*128, per-(map,head) padded feat block
DIMS = 6 * PADMH  # 29952
OS = 512 // NCORES  # 64 output cols per core
F32 = mybir.dt.float32
BF16 = mybir.dt.bfloat16
AX = mybir.ActivationFunctionType
OP = mybir.AluOpType

ZC_STATIONARY = True  # col sums via 1-col matmuls (E stationary) vs streaming ones-matmul


def _host_prep(ln_w, ln_b, qkv_w, proj_w, proj_b):
    ln_w = ln_w.astype(np.float64)
    ln_b = ln_b.astype(np.float64)
    qw = qkv_w.astype(np.float64)
    Wp = qw * ln_w[None, :]  # [576, C]
    r = Wp.sum(axis=1)  # [576]
    t = qw @ ln_b  # [576]

    # per-side packing: side0 tiles hold [k_h; q_h], side1 [q_h; k_h] so the
    # attention matmul operands always share a partition base
    def col(fsl, scale):
        return np.concatenate([Wp[fsl] * scale, (r[fsl] * scale)[:, None],
                               (t[fsl] * scale)[:, None]], axis=1).T

    wqk = np.zeros((2, C + 2, 3 * 128), np.float32)
    for h in range(H):
        qr = slice(h * HD, (h + 1) * HD)
        kr = slice(F + h * HD, F + (h + 1) * HD)
        qcols = col(qr, SCALE)  # [C+2, 64]
        kcols = col(kr, 1.0)
        wqk[0, :, h * 128 : h * 128 + 64] = kcols
        wqk[0, :, h * 128 + 64 : h * 128 + 128] = qcols
        wqk[1, :, h * 128 : h * 128 + 64] = qcols
        wqk[1, :, h * 128 + 64 : h * 128 + 128] = kcols
    wqk = wqk.astype(ml_dtypes.bfloat16)

    wv = np.zeros((C + 2, F), np.float32)
    wv[:C] = Wp[2 * F :].T
    wv[C] = r[2 * F :]
    wv[C + 1] = t[2 * F :]
    wv = wv.astype(ml_dtypes.bfloat16)

    ys = np.linspace(-1.0, 1.0, HG)
    xs = np.linspace(-1.0, 1.0, WG)
    p3 = np.tile(ys, WG)
    p4 = np.repeat(xs, HG)
    pos = np.stack([p3 * p3, p4 * p4, p3 * p4, p3, p4, np.ones_like(p3)], axis=1)
    pos_pad = np.zeros((640, 6), np.float32)
    pos_pad[:N] = pos

    pwt = np.zeros((DIMS, 512), np.float32)
    for mh in range(6):
        blk = proj_w[:, mh * 4900 : (mh + 1) * 4900]  # [512, 4900]
        pwt[mh * PADMH : mh * PADMH + 4900] = blk.T
    pwt = pwt.astype(ml_dtypes.bfloat16)
    sel8 = np.zeros((BP, BP, 128), ml_dtypes.bfloat16)
    for i in range(BP):
        sel8[i, i, :] = 1.0
    return wqk, wv, pos_pad, pwt, sel8


def _build():
    nc = bacc.Bacc()
    x1d = nc.declare_dram_parameter("x1s", [BP, C, N], BF16, isOutput=False)
    x2d = nc.declare_dram_parameter("x2s", [BP, C, N], BF16, isOutput=False)
    wqkd = nc.declare_dram_parameter("wqk", [2, C + 2, 3 * 128], BF16, isOutput=False)
    wvd = nc.declare_dram_parameter("wv", [C + 2, F], BF16, isOutput=False)
    posd = nc.declare_dram_parameter("pos", [640, 6], F32, isOutput=False)
    pwtd = nc.declare_dram_parameter("pwt", [DIMS, OS], BF16, isOutput=False)
    pbd = nc.declare_dram_parameter("pb", [1, OS], F32, isOutput=False)
    sel8d = nc.declare_dram_parameter("sel8", [BP, BP, 128], BF16, isOutput=False)
    outd = nc.declare_dram_parameter("out", [B, OS], F32, isOutput=True)
    statsd = nc.dram_tensor("statsd", [2, 2, BP, N], BF16)  # (negmu, sigma)
    feat8d = [nc.dram_tensor(f"feat8_{j}", [BP, PADMH], BF16) for j in range(6)]
    featAG = [
        nc.dram_tensor(f"featAG_{j}", [B, PADMH], BF16, addr_space="Shared")
        for j in range(6)
    ]
    xd = [x1d, x2d]

    def bcast_p(sl, p):
        return bass.AP(tensor=sl.tensor, offset=sl.offset, ap=[[0, p]] + list(sl.ap))

    with ExitStack() as ctx:
        tc = ctx.enter_context(tile.TileContext(nc))
        const = ctx.enter_context(tc.tile_pool(name="const", bufs=1))
        xres = ctx.enter_context(tc.tile_pool(name="xres", bufs=1))
        stats = ctx.enter_context(tc.tile_pool(name="stats", bufs=1))
        tmp = ctx.enter_context(tc.tile_pool(name="tmp", bufs=2))
        sb_qk = ctx.enter_context(tc.tile_pool(name="sbqk", bufs=1))
        sb_vp = ctx.enter_context(tc.tile_pool(name="sbvp", bufs=1))
        epool = ctx.enter_context(tc.tile_pool(name="epool", bufs=11))
        e2pool = ctx.enter_context(tc.tile_pool(name="e2pool", bufs=11))
        zpool = ctx.enter_context(tc.tile_pool(name="zpool", bufs=3))
        upool = ctx.enter_context(tc.tile_pool(name="upool", bufs=8))
        fpool = ctx.enter_context(tc.tile_pool(name="fpool", bufs=3))
        ftpool = ctx.enter_context(tc.tile_pool(name="ftpool", bufs=3))
        opool = ctx.enter_context(tc.tile_pool(name="opool", bufs=2))
        # PSUM: tag pA [128,576]x2 = 4 banks; tag pU [128,192]x4 = 4 banks
        psA = ctx.enter_context(tc.tile_pool(name="psA", bufs=2, space="PSUM"))
        psU = ctx.enter_context(tc.tile_pool(name="psU", bufs=4, space="PSUM"))

        # ---- constants ----
        wqk_sb = [[], []]
        wv_sb = []
        for k, (k0, kw) in enumerate([(0, 128), (128, 128), (256, 2)]):
            for s in range(2):
                wt = const.tile([kw, 3 * 128], BF16, tag=f"wqk{s}_{k}")
                nc.sync.dma_start(out=wt, in_=wqkd[s, k0 : k0 + kw, :])
                wqk_sb[s].append(wt)
            vt = const.tile([kw, F], BF16, tag=f"wv{k}")
            nc.sync.dma_start(out=vt, in_=wvd[k0 : k0 + kw, :])
            wv_sb.append(vt)
        pos_sb = const.tile([128, 5, 6], F32, tag="pos")
        nc.sync.dma_start(out=pos_sb, in_=posd.rearrange("(t p) e -> p t e", p=128))
        ind8_sb = const.tile([128, BP, BP], BF16, tag="ind8")
        nc.vector.memset(ind8_sb, 0.0)
        for i in range(BP):
            nc.vector.memset(ind8_sb[:, i, i : i + 1], 1.0)
        # per-item one-hot selector for K=8 broadcast matmuls (bf16 so the
        # broadcast matmul runs at 1 cyc/row; rhs stats tiles are bf16 too)
        sel8_sb = const.tile([BP, BP, 128], BF16, tag="sel8")
        nc.sync.dma_start(out=sel8_sb, in_=sel8d[:])
        onesb_sb = const.tile([128, 32], BF16, tag="onesb")
        nc.vector.memset(onesb_sb, 1.0)
        epssb = const.tile([32, 1], F32, tag="eps")
        nc.vector.memset(epssb, EPS)
        pb_sb = const.tile([B, OS], F32, tag="pb")
        nc.gpsimd.dma_start(out=pb_sb, in_=bcast_p(pbd[0, :], B))
        zpad = const.tile([1, 552], BF16, tag="zpad")
        nc.vector.memset(zpad, 0.0)
        for j in range(6):
            for ib in range(BP):
                nc.gpsimd.dma_start(
                    out=feat8d[j][ib, 4900:PADMH],
                    in_=bass.AP(tensor=zpad.tensor, offset=zpad.offset, ap=[[1, 1], [1, 92]]),
                )

        # ---- phase 1a: LN stats (x streamed on sync/gpsimd queues) ----
        qdma = [nc.sync, nc.gpsimd]
        isv_sb = []
        isColT = []
        for s in range(2):
            psum_s = psA.tile([128, N], F32, tag="pA")
            psum_q = psA.tile([128, N], F32, tag="pA")
            for i in range(BP):
                for k in range(2):
                    xt = xres.tile([128, N], BF16, tag="x", bufs=6, name="xt")
                    qdma[(i + k) % 2].dma_start(
                        out=xt, in_=xd[s][i, k * 128 : (k + 1) * 128, :]
                    )
                    xq = tmp.tile([128, N], BF16, tag="xsq")
                    nc.vector.tensor_mul(xq, xt, xt)
                    st = i == 0 and k == 0
                    for c0, cw in NCH:
                        nc.tensor.matmul(
                            psum_s[:BP, c0 : c0 + cw], ind8_sb[:, i, :], xt[:, c0 : c0 + cw],
                            start=st, stop=(i == BP - 1 and k == 1),
                        )
                        nc.tensor.matmul(
                            psum_q[:BP, c0 : c0 + cw], ind8_sb[:, i, :], xq[:, c0 : c0 + cw],
                            start=st, stop=(i == BP - 1 and k == 1),
                        )
            mean = stats.tile([32, N], F32, tag="mean")
            ex2 = stats.tile([32, N], F32, tag="ex2")
            nc.vector.tensor_scalar_mul(mean[:BP], psum_s[:BP], 1.0 / C)
            nc.vector.tensor_scalar_mul(ex2[:BP], psum_q[:BP], 1.0 / C)
            var = stats.tile([32, N], F32, tag="var")
            nc.vector.scalar_tensor_tensor(
                out=var[:BP], in0=mean[:BP], scalar=-1.0, in1=mean[:BP], op0=OP.mult, op1=OP.mult
            )
            nc.vector.tensor_add(var[:BP], var[:BP], ex2[:BP])
            sig = stats.tile([32, N], F32, tag="sig")
            nc.scalar.activation(out=sig[:BP], in_=var[:BP], func=AX.Sqrt, bias=epssb[:BP])
            isvf = stats.tile([32, N], F32, tag=f"isvf{s}")
            nc.vector.reciprocal(isvf[:BP], sig[:BP])
            isv = stats.tile([32, N], BF16, tag=f"isv{s}")
            nc.vector.tensor_copy(isv[:BP], isvf[:BP])
            negmu = stats.tile([32, N], BF16, tag="negmu")
            nc.vector.tensor_scalar_mul(negmu[:BP], mean[:BP], -1.0)
            sigb = stats.tile([32, N], BF16, tag="sigb")
            nc.vector.tensor_copy(sigb[:BP], sig[:BP])
            nc.sync.dma_start(out=statsd[s, 0], in_=negmu[:BP])
            nc.sync.dma_start(out=statsd[s, 1], in_=sigb[:BP])
            # per-(tile,item) 1/sigma columns for the v scale
            zt_is = stats.tile([32, 18, 32], F32, tag="zt_is")
            nc.vector.transpose(out=zt_is, in_=isvf.rearrange("p (g q) -> p g q", q=32))
            ict = const.tile([128, 5, BP], F32, tag=f"iscol{s}")
            for a in range(4):
                ng = 5 if a < 2 else 4
                nc.vector.tensor_copy(
                    out=ict[32 * a : 32 * a + 32, 0:ng, :],
                    in_=zt_is[:, a : 18 : 4, 0:BP],
                )
            isv_sb.append(isv)
            isColT.append(ict)

        # ---- phase 1b: QKV for all items (1-step skewed emission) ----
        qs = {}
        ks = {}
        vp = {}
        for i in range(BP):
            for s in range(2):
                xe = stats.tile([2, N], BF16, tag="xe", bufs=3, name="xe")
                nc.sync.dma_start(out=xe, in_=statsd[s, :, i, :])
                xt0 = xres.tile([128, N], BF16, tag="x", bufs=6, name="xt0")
                qdma[i % 2].dma_start(out=xt0, in_=xd[s][i, 0:128, :])
                xt1 = xres.tile([128, N], BF16, tag="x", bufs=6, name="xt1")
                qdma[(i + 1) % 2].dma_start(out=xt1, in_=xd[s][i, 128:256, :])
                # broadcast 1/sigma row across 128 partitions via K=8 matmul
                pis = psA.tile([128, N], F32, tag="pA")
                for c0, cw in NCH:
                    nc.tensor.matmul(
                        pis[:, c0 : c0 + cw], sel8_sb[:, i, :], isv_sb[s][0:BP, c0 : c0 + cw],
                        start=True, stop=True,
                    )
                isb = tmp.tile([128, N], F32, tag="isb")
                nc.vector.tensor_copy(out=isb, in_=pis)
                rhs3 = [xt0, xt1, xe]
                pqs = []
                for h in range(H):
                    pq = psA.tile([128, N], F32, tag="pA")
                    for k in range(3):
                        for c0, cw in NCH:
                            nc.tensor.matmul(
                                pq[:, c0 : c0 + cw],
                                wqk_sb[s][k][:, h * 128 : (h + 1) * 128],
                                rhs3[k][:, c0 : c0 + cw],
                                start=(k == 0), stop=(k == 2),
                            )
                    pqs.append(pq)
                    if h >= 1:  # consume pq(h-1) so the pA pool never blocks
                        _qk_emit(nc, sb_qk, qs, ks, pqs[h - 1], isb, i, s, h - 1)
                pvs = []
                for nt in range(5):
                    w = NT[nt]
                    n0 = nt * 128
                    pv = psU.tile([128, F], F32, tag="pU")
                    for k in range(3):
                        nc.tensor.matmul(
                            pv[:w],
                            rhs3[k][:, n0 : n0 + w],
                            wv_sb[k],
                            start=(k == 0), stop=(k == 2),
                        )
                    pvs.append(pv)
                    if nt == 0:
                        _qk_emit(nc, sb_qk, qs, ks, pqs[2], isb, i, s, 2)
                    if nt >= 2:
                        _vt_emit(nc, sb_vp, vp, pos_sb, isColT, pvs[nt - 2], i, s, nt - 2)
                _vt_emit(nc, sb_vp, vp, pos_sb, isColT, pvs[3], i, s, 3)
                _vt_emit(nc, sb_vp, vp, pos_sb, isColT, pvs[4], i, s, 4)

        # ---- phase 2: pipelined attention; chunked AllGather + proj overlap ----
        oacc = opool.tile([B, OS], F32, tag="oacc")
        nc.vector.memset(oacc, 0.0)

        def emit_proj(mh):
            GSZ = 13
            for g0 in range(0, 39, GSZ):
                ft = ftpool.tile([128, GSZ, B], BF16, tag="ft")
                nc.sync.dma_start_transpose(
                    out=ft, in_=featAG[mh][:, g0 * 128 : (g0 + GSZ) * 128]
                )
                pw = ftpool.tile([128, GSZ, OS], BF16, tag="pw")
                nc.gpsimd.dma_start(
                    out=pw,
                    in_=pwtd[mh * PADMH + g0 * 128 : mh * PADMH + (g0 + GSZ) * 128, :]
                    .rearrange("(j p) o -> p j o", p=128),
                )
                opsum = psA.tile([64, OS], F32, tag="pA")
                for j in range(GSZ):
                    nc.tensor.matmul(
                        opsum, ft[:, j, :], pw[:, j, :],
                        start=(j == 0), stop=(j == GSZ - 1),
                    )
                nc.vector.tensor_add(oacc, oacc, opsum)

        def emit_gather(j):
            nc.gpsimd.collective_compute(
                "AllGather",
                OP.bypass,
                replica_groups=[list(range(NCORES))],
                ins=[feat8d[j][:]],
                outs=[featAG[j][:]],
            )

        blocks = [(m, h, i) for m in range(2) for h in range(H) for i in range(BP)]
        proj_at = {3: [0, 1], 5: [2, 3]}  # keyed on mh at block start (i==0)
        gather_at = {1: [0, 1], 3: [2, 3], 4: [4]}  # keyed on mh after last item

        class Blk:
            __slots__ = ("m", "h", "i", "et", "e2", "zr5", "rzr5", "zcp", "rzc", "fps")

        def emit_scores(b):
            """QK^T + exp for block b; returns ctx with et/zr5. Emits the
            first two row-tiles; remaining tiles emitted by emit_scores_rest
            interleaved with the previous block's consumers."""
            m, h, i = b.m, b.h, b.i
            qside = 1 - m
            b.zr5 = zpool.tile([128, 8], F32, tag="zr5")
            b.et = []
            for nt in range(5):
                w = NT[nt]
                n0 = nt * 128
                pa = psA.tile([128, N], F32, tag="pA")
                for c0, cw in NCH:
                    nc.tensor.matmul(
                        pa[:w, c0 : c0 + cw],
                        qs[i, qside, h][:, n0 : n0 + w],
                        ks[i, m, h][:, c0 : c0 + cw],
                        start=True, stop=True,
                    )
                et = epool.tile([128, N], BF16, tag="E")
                nc.scalar.activation(
                    out=et[:w], in_=pa[:w], func=AX.Exp,
                    accum_out=b.zr5[:w, nt : nt + 1],
                )
                b.et.append(et)
                yield nt

        def emit_zc(b):
            # column sums of E via 1-col matmuls with E stationary
            b.zcp = psU.tile([128, 8], F32, tag="pU")
            for mc in range(5):
                w2 = NT[mc]
                for nt in range(5):
                    w = NT[nt]
                    nc.tensor.matmul(
                        b.zcp[:w2, mc : mc + 1],
                        b.et[nt][:w, mc * 128 : mc * 128 + w2],
                        onesb_sb[:w, 0:1],
                        start=(nt == 0), stop=(nt == 4),
                    )

        def emit_upf(p):
            # up/us/f chain for the previous block p (e2/rzc ready)
            vside = p.m
            h, i = p.h, p.i
            p.fps = psU.tile([128, 72], F32, tag="pU")
            ups = []
            uss = []

            def one_up(mc):
                w2 = NT[mc]
                up = psU.tile([128, 72], F32, tag="pU")
                for nt in range(5):
                    w = NT[nt]
                    nc.tensor.matmul(
                        up[:w2, 0:70],
                        p.e2[nt][:w, mc * 128 : mc * 128 + w2],
                        vp[i, vside, nt][:w, h, 0:70],
                        start=(nt == 0), stop=(nt == 4),
                    )
                ups.append(up)

            def one_us(mc):
                w2 = NT[mc]
                us = upool.tile([128, 72], BF16, tag="us")
                nc.vector.tensor_scalar_mul(
                    us[:w2, 0:70], ups[mc][:w2, 0:70], p.rzc[:w2, mc : mc + 1]
                )
                uss.append(us)

            def one_f(mc):
                w2 = NT[mc]
                nc.tensor.matmul(
                    p.fps[0:70, 0:70],
                    uss[mc][:w2, 0:70],
                    vp[i, vside, mc][:w2, h, 0:70],
                    start=(mc == 0), stop=(mc == 4),
                )

            one_up(0)
            one_us(0)
            one_up(1)
            one_us(1)
            one_f(0)
            one_up(2)
            one_us(2)
            one_f(1)
            one_up(3)
            one_us(3)
            one_f(2)
            one_up(4)
            one_us(4)
            one_f(3)
            one_f(4)

        def emit_fstore(p):
            mh = p.m * 3 + p.h
            fb = fpool.tile([70, 70], BF16, tag="fb")
            nc.vector.tensor_copy(out=fb, in_=p.fps[0:70, 0:70])
            nc.sync.dma_start(
                out=feat8d[mh][p.i, 0:4900].rearrange("(d e) -> d e", e=70),
                in_=fb,
            )

        prev = None
        for m, h, i in blocks:
            mh = m * 3 + h
            if i == 0:
                for pj in proj_at.get(mh, []):
                    emit_proj(pj)
            b = Blk()
            b.m, b.h, b.i = m, h, i
            gen = emit_scores(b)
            next(gen)  # pa/exp nt=0
            next(gen)  # pa/exp nt=1
            if prev is not None:
                emit_zc(prev)  # PE: dep only on et(prev) -- ready
            next(gen)  # nt=2
            if prev is not None:
                # rzc for prev now that its zc matmuls are queued
                prev.rzc = zpool.tile([128, 8], F32, tag="rzc")
                nc.vector.reciprocal(prev.rzc[:, 0:5], prev.zcp[:, 0:5])
            next(gen)  # nt=3
            for _ in gen:  # nt=4
                pass
            if prev is not None:
                emit_upf(prev)
                emit_fstore(prev)
                pmh = prev.m * 3 + prev.h
                if prev.i == BP - 1:
                    for j in gather_at.get(pmh, []):
                        emit_gather(j)
            # end-of-step DVE for b: 1/Zr then E^2/Zr tiles
            b.rzr5 = zpool.tile([128, 8], F32, tag="rzr5")
            nc.vector.reciprocal(b.rzr5[:, 0:5], b.zr5[:, 0:5])
            b.e2 = []
            for nt in range(5):
                w = NT[nt]
                e2 = e2pool.tile([128, N], BF16, tag="E2")
                nc.vector.scalar_tensor_tensor(
                    out=e2[:w], in0=b.et[nt][:w], scalar=b.rzr5[:w, nt : nt + 1],
                    in1=b.et[nt][:w], op0=OP.mult, op1=OP.mult,
                )
                b.e2.append(e2)
            prev = b

        # flush last block
        emit_zc(prev)
        prev.rzc = zpool.tile([128, 8], F32, tag="rzc")
        nc.vector.reciprocal(prev.rzc[:, 0:5], prev.zcp[:, 0:5])
        emit_upf(prev)
        emit_fstore(prev)
        for j in gather_at.get(5, []):
            emit_gather(j)

        emit_proj(4)
        emit_gather(5)
        emit_proj(5)
        osb = opool.tile([B, OS], F32, tag="osb")
        nc.vector.tensor_add(osb, oacc, pb_sb)
        nc.vector.tensor_scalar_max(osb, osb, 0.0)
        nc.sync.dma_start(out=outd[:], in_=osb)

    nc.compile()
    return nc


def _qk_emit(nc, sb_qk, qs, ks, pq, isb, i, s, h):
    qk = sb_qk.tile([128, N], BF16, tag=f"qk{i}_{s}_{h}", name="qk")
    nc.vector.tensor_mul(qk, pq, isb)
    if s == 0:
        ks[i, s, h] = qk[0:64, :]
        qs[i, s, h] = qk[64:128, :]
    else:
        qs[i, s, h] = qk[0:64, :]
        ks[i, s, h] = qk[64:128, :]


def _vt_emit(nc, sb_vp, vp, pos_sb, isColT, pv, i, s, nt):
    w = NT[nt]
    vt = sb_vp.tile([128, 3, 72], mybir.dt.bfloat16, tag=f"vp{i}_{s}_{nt}", name="vt")
    nc.vector.tensor_scalar_mul(
        vt[:w, :, 0:64],
        pv[:w, 0:F].rearrange("p (a b) -> p a b", b=64),
        isColT[s][:w, nt, i : i + 1],
    )
    ps = pos_sb[:w, nt, :]
    nc.vector.tensor_copy(
        out=vt[:w, :, 64:70],
        in_=bass.AP(tensor=ps.tensor, offset=ps.offset,
                    ap=[ps.ap[0], [0, 3], ps.ap[-1]]),
    )
    vp[i, s, nt] = vt


def kernel(x1, x2, ln_w, ln_b, qkv_w, proj_w, proj_b):
    wqk, wv, pos_pad, pwt, sel8 = _host_prep(ln_w, ln_b, qkv_w, proj_w, proj_b)
    xs1 = np.ascontiguousarray(x1.reshape(B, C, N)).astype(ml_dtypes.bfloat16)
    xs2 = np.ascontiguousarray(x2.reshape(B, C, N)).astype(ml_dtypes.bfloat16)
    nc = _build()
    in_maps = []
    for r in range(NCORES):
        in_maps.append(
            {
                "x1s": xs1[r * BP : (r + 1) * BP],
                "x2s": xs2[r * BP : (r + 1) * BP],
                "wqk": wqk,
                "wv": wv,
                "pos": pos_pad,
                "pwt": np.ascontiguousarray(pwt[:, r * OS : (r + 1) * OS]),
                "pb": np.ascontiguousarray(proj_b[None, r * OS : (r + 1) * OS]).astype(np.float32),
                "sel8": sel8,
            }
        )
    import os

    trace = bool(os.environ.get("BASS_TRACE"))
    res = run_bass_kernel_spmd(nc, in_maps, core_ids=list(range(NCORES)), trace=trace)
    if res.exec_time_ns is not None:
        print(f"HW exec time: {res.exec_time_ns} ns")
    if res.instructions_and_trace:
        print("trace path:", res.instructions_and_trace[1])
    out = np.concatenate([res.results[r]["out"] for r in range(NCORES)], axis=1)
    return out.astype(np.float32)


if __name__ == "__main__":
    rng = np.random.default_rng(0)
    ins = {
        "x1": rng.standard_normal((B, C, HG, WG), dtype=np.float32),
        "x2": rng.standard_normal((B, C, HG, WG), dtype=np.float32),
        "ln_w": np.ones(C, np.float32),
        "ln_b": np.zeros(C, np.float32),
        "qkv_w": (rng.standard_normal((3 * F, C)) * C**-0.5).astype(np.float32),
        "proj_w": (rng.standard_normal((512, 6 * 4900)) * (6 * 4900) ** -0.5).astype(np.float32),
        "proj_b": np.zeros(512, np.float32),
    }
    print(kernel(**ins).shape)
